# revision 23
# baseline (speedup 1.0000x reference)
"""VQ codebook lookup kernel for Trainium2 (8 NeuronCores, data-parallel).

out[b] = emb[argmin_k ||x[b] - emb[k]||^2]

Per core (8192 rows of x), per 128-row tile:
  score[b,k] = 2*x.e_k - |e_k|^2 (argmax == argmin of distance), computed as
    P1  = xh.eh              fp16 x fp16   (hi parts; 4 col-passes)
    P2  = (x/32).(32*el)     fp8  DoubleRow (x-lo correction vs e-hi residual)
    P3  = (256*xl).(eh/256)  fp8  DoubleRow
    bias= ones2.(q/2 splits) fp8  DoubleRow (4-way fp8 cascade of -|e_k|^2)
  fp8 e4m3 DoubleRow packs 256 contraction dims per instr at 0.5 cyc/col,
  so the correction passes cost 1/4 of the fp16 equivalent. Scales keep the
  fp8 operands inside e4m3's normal range; products are scale-free.
  argmax: DVE tensor_tensor_reduce (max of the two 512-wide PSUM halves,
  fused reduce -> per-row max) + one find_index8 over the full PSUM scores.
  Gather: emb rows fetched with indirect DMA batched GATHER_T tiles per
  instr (SWDGE issue cost is ~1us fixed per instr); stores batched the same.
"""
import os
import sys

import numpy as np
import ml_dtypes

for _p in ("/opt/trn_rl_repo", "/root/.axon_site/_ro/trn_rl_repo"):
    if os.path.isdir(_p) and _p not in sys.path:
        sys.path.append(_p)

import concourse.bass as bass
import concourse.tile as tile
from concourse import bacc, mybir
from concourse.bass_utils import run_bass_kernel_spmd

N_CORES = 8
B, D, K = 65536, 256, 1024
BC = B // N_CORES            # rows per core
TILE_B = 128
N_TILES = BC // TILE_B       # 64
GATHER_T = 4                 # tiles per batched gather/store
A_SC = 5                     # x/2^a vs el*2^a fp8 scaling
B_SC = 8                     # xl*2^b vs eh/2^b fp8 scaling
# column chunking of the x loads: small first chunk so the PE starts early
CHUNK_BOUNDS = [0, 256, 1024, 2048, 3072, 4096, 5120, 6144, 7168, 8192]

f32 = mybir.dt.float32
f16 = mybir.dt.float16
f8 = mybir.dt.float8e4
f8e5 = mybir.dt.float8e5
u32 = mybir.dt.uint32
np_f8 = ml_dtypes.float8_e4m3
np_f8e5 = ml_dtypes.float8_e5m2

_nc_cache = {}


def _build():
    if "nc" in _nc_cache:
        return _nc_cache["nc"]
    nc = bacc.Bacc()

    xh0 = nc.declare_dram_parameter("xh0", [128, BC], f16, isOutput=False)
    xh1 = nc.declare_dram_parameter("xh1", [128, BC], f16, isOutput=False)
    # x-side fp8 weights in DoubleRowSwInterleave layout: per 128-col tile,
    # [A127,B127,...,A0,B0] (A/B = k-tile 0/1, columns reversed)
    xq8 = nc.declare_dram_parameter("xq8", [128, 2 * BC], f8e5, isOutput=False)
    xl8 = nc.declare_dram_parameter("xl8", [128, 2 * BC], f8e5, isOutput=False)
    eh0 = nc.declare_dram_parameter("eh0", [128, K], f16, isOutput=False)
    eh1 = nc.declare_dram_parameter("eh1", [128, K], f16, isOutput=False)
    el8 = nc.declare_dram_parameter("el8", [128, 2, K], f8e5, isOutput=False)
    eh8 = nc.declare_dram_parameter("eh8", [128, 2, K], f8e5, isOutput=False)
    bx = nc.declare_dram_parameter("bx", [128, 128], f16, isOutput=False)
    bq = nc.declare_dram_parameter("bq", [128, K], f16, isOutput=False)
    emb = nc.declare_dram_parameter("emb", [K, D], f32, isOutput=False)
    out = nc.declare_dram_parameter("out", [BC, D], f32, isOutput=True)

    xsrc = {"xh0": xh0, "xh1": xh1, "xq8": xq8, "xl8": xl8}
    DR = mybir.MatmulPerfMode.DoubleRowSwInterleave

    with tile.TileContext(nc) as tc:
        with tc.tile_pool(name="res", bufs=1) as res, \
             tc.tile_pool(name="scr", bufs=2) as scr, \
             tc.tile_pool(name="idx", bufs=2) as idxp, \
             tc.tile_pool(name="gat", bufs=2) as gat, \
             tc.tile_pool(name="ps", bufs=4, space="PSUM") as ps:
            teh0 = res.tile([128, K], f16, tag="eh0")
            teh1 = res.tile([128, K], f16, tag="eh1")
            tel8 = res.tile([128, 2, K], f8e5, tag="el8")
            teh8 = res.tile([128, 2, K], f8e5, tag="eh8")
            tbx = res.tile([128, 128], f16, tag="bx")
            tbq = res.tile([128, K], f16, tag="bq")
            # persistent max buffer: slot 0 rewritten per tile (DVE is
            # in-order so reuse is race-free); slots 1-7 are never-matching
            # fillers for find_index8's 8-wide match register
            tmax8 = res.tile([128, 8], f32, tag="maxv")
            nc.vector.memset(tmax8[:], 0)

            xch = {}

            def load_chunk(nm, j):
                lo, hi = CHUNK_BOUNDS[j], CHUNK_BOUNDS[j + 1]
                if nm in ("xq8", "xl8"):
                    t = res.tile([128, 2 * (hi - lo)], f8e5, tag=f"{nm}_{j}")
                    nc.sync.dma_start(t[:], xsrc[nm][:, 2 * lo:2 * hi])
                else:
                    t = res.tile([128, hi - lo], f16, tag=f"{nm}_{j}")
                    nc.sync.dma_start(t[:], xsrc[nm][:, lo:hi])
                xch[(nm, j)] = t

            # load order mirrors the in-tile matmul order so the PE can
            # start as soon as the first tensors land
            load_chunk("xh0", 0)
            nc.sync.dma_start(teh0[:], eh0[:])
            load_chunk("xh1", 0)
            nc.sync.dma_start(teh1[:], eh1[:])
            load_chunk("xq8", 0)
            nc.sync.dma_start(tel8[:], el8[:])
            load_chunk("xl8", 0)
            nc.sync.dma_start(teh8[:], eh8[:])
            nc.sync.dma_start(tbx[:], bx[:])
            nc.sync.dma_start(tbq[:], bq[:])
            for j in range(1, len(CHUNK_BOUNDS) - 1):
                for nm in ("xh0", "xh1", "xq8", "xl8"):
                    load_chunk(nm, j)

            def col(i):
                c0 = i * TILE_B
                for j in range(len(CHUNK_BOUNDS) - 1):
                    if c0 < CHUNK_BOUNDS[j + 1]:
                        return j, c0 - CHUNK_BOUNDS[j]
                raise AssertionError

            tibuf = None
            for i in range(N_TILES):
                j, c0 = col(i)
                s = slice(c0, c0 + TILE_B)
                cxh0 = xch[("xh0", j)][:, s]
                cxh1 = xch[("xh1", j)][:, s]
                s2 = slice(2 * c0, 2 * c0 + 256)
                cxq8 = xch[("xq8", j)][:, s2]
                cxl8 = xch[("xl8", j)][:, s2]

                psc = ps.tile([128, K], f32, tag="scores")
                mm = nc.tensor.matmul
                # fp8 DoubleRow instrs FIRST: their PSUM read-modify-write
                # has ~2^-12-relative noise, so they must only ever touch a
                # small-magnitude partial sum. The exact fp16 instrs (incl.
                # the 3-row bias cascade, zero-padded to 128 rows) follow.
                for h in range(2):
                    hs = psc[:, h * 512:(h + 1) * 512]
                    ks = slice(h * 512, (h + 1) * 512)
                    mm(hs, lhsT=cxq8, rhs=tel8[:, :, ks], start=True,
                       stop=False, perf_mode=DR)
                    mm(hs, lhsT=cxl8, rhs=teh8[:, :, ks], start=False,
                       stop=False, perf_mode=DR)
                    mm(hs, lhsT=cxh0, rhs=teh0[:, ks], start=False, stop=False)
                    mm(hs, lhsT=cxh1, rhs=teh1[:, ks], start=False, stop=False)
                    mm(hs, lhsT=tbx[:], rhs=tbq[:, ks], start=False, stop=True)

                # per-row top-8 (the custom-DVE fused reduce crashes hw here,
                # so plain MAX8 it is)
                nc.vector.max(out=tmax8[:], in_=psc[:])

                tidx = idxp.tile([128, 8], u32, tag="idx")
                nc.vector.max_index(
                    out=tidx[:],
                    in_max=tmax8[:],
                    in_values=psc[:],
                )

                tg = gat.tile([128, D], f32, tag="gat")
                nc.gpsimd.indirect_dma_start(
                    out=tg[:],
                    out_offset=None,
                    in_=emb[:],
                    in_offset=bass.IndirectOffsetOnAxis(ap=tidx[:, 0:1], axis=0),
                )
                nc.scalar.dma_start(out[i * TILE_B:(i + 1) * TILE_B, :], tg[:])

    nc.compile()
    _nc_cache["nc"] = nc
    return nc


def _prepare_inputs(x, emb):
    x = np.ascontiguousarray(np.asarray(x, dtype=np.float32))
    emb = np.ascontiguousarray(np.asarray(emb, dtype=np.float32))

    e2 = np.ascontiguousarray(2.0 * emb.T)              # [D, K] f32, exact
    eh = e2.astype(np.float16)                          # fp16 hi
    el = (e2 - eh.astype(np.float32))                   # f32 residual

    def fp8s(v, s):  # fp8 of v * 2^s
        return np.ldexp(v, s).astype(np_f8)

    def pack_mov(v8):  # [256, N] -> [128, 2, N] (contraction = t*128 + p)
        return np.ascontiguousarray(
            v8.reshape(2, 128, v8.shape[1]).transpose(1, 0, 2))

    def pack_swi(v8):
        # [256, N] -> [128, 2N] DoubleRowSwInterleave weights: per 128-col
        # tile, free = [A127,B127,A126,B126,...,A0,B0] (A/B = ktile 0/1,
        # columns reversed within the tile)
        n = v8.shape[1]
        nt = n // 128
        a = v8[:128].reshape(128, nt, 128)[:, :, ::-1]   # [p, tile, m-rev]
        b = v8[128:].reshape(128, nt, 128)[:, :, ::-1]
        inter = np.stack([a, b], axis=-1)                # [p, tile, m, 2]
        return np.ascontiguousarray(inter.reshape(128, 2 * n))

    el8 = pack_mov(fp8s(el, A_SC).astype(np_f8e5))      # 32*el
    eh8 = pack_mov(
        fp8s(eh.astype(np.float32), -B_SC).astype(np_f8e5))  # eh/256

    # 3-way fp16 cascade of the -|e_k|^2 bias, via an fp16 matmul with
    # ones rows (zero-padded to 128 partitions; cost is per-column anyway)
    q = -(emb.astype(np.float64) ** 2).sum(axis=1)
    qs = []
    r = q.copy()
    for _ in range(3):
        qi = r.astype(np.float32).astype(np.float16)
        qs.append(qi)
        r = r - qi.astype(np.float64)
    bq = np.zeros((128, K), np.float16)
    bq[0], bq[1], bq[2] = qs
    bx = np.zeros((128, 128), np.float16)
    bx[0:3, :] = 1.0

    xT = np.ascontiguousarray(x.T)                      # [D, B] f32
    xh = xT.astype(np.float16)
    xl = xT - xh.astype(np.float32)
    xq8_full = pack_swi(fp8s(xT, -A_SC).astype(np_f8e5))   # x/32
    xl8_full = pack_swi(fp8s(xl, B_SC).astype(np_f8e5))     # 256*xl

    in_maps = []
    for c in range(N_CORES):
        sl = slice(c * BC, (c + 1) * BC)
        in_maps.append({
            "xh0": np.ascontiguousarray(xh[:128, sl]),
            "xh1": np.ascontiguousarray(xh[128:, sl]),
            "xq8": np.ascontiguousarray(xq8_full[:, 2 * sl.start:2 * sl.stop]),
            "xl8": np.ascontiguousarray(xl8_full[:, 2 * sl.start:2 * sl.stop]),
            "eh0": np.ascontiguousarray(eh[:128]),
            "eh1": np.ascontiguousarray(eh[128:]),
            "el8": el8,
            "eh8": eh8,
            "bx": bx,
            "bq": bq,
            "emb": emb,
        })
    return in_maps


def run(x, emb, trace=False, **kwargs):
    """Run the kernel; returns (out, BassKernelResults)."""
    nc = _build()
    in_maps = _prepare_inputs(x, emb)
    res = run_bass_kernel_spmd(nc, in_maps, list(range(N_CORES)),
                               trace=trace, **kwargs)
    out = np.concatenate([res.results[c]["out"] for c in range(N_CORES)], axis=0)
    return out, res


def kernel(x, emb):
    out, _ = run(x, emb, trace=False)
    return out
